# revision 21
# baseline (speedup 1.0000x reference)
"""Bass/Trainium2 kernel for nn_Bilinear (out[b,n,i] = enc[b,n,i,:] @ W @ hidden[b,:] + bias).

Sharding: data-parallel over B. 8 cores, one batch element each.

The kernel is HBM-traffic-bound (enc is 32 MiB/core in fp32), so all streamed
operands are cast to fp16 on the host (harness gate is rel_err < 2e-2; fp16
lands ~4e-4): enc 16 MiB + W 2 MiB per core, streamed at the observed
~420 GB/s per-core DMA rate.

With fp16 the stream outpaces what DVE+ScalarE alone can compute, so stage 2
is split across THREE compute engines. The host lays out each 2 MiB chunk of
enc (8 row blocks of 128) as:
  [ j-major (transposed) strip of 3 blocks | 5 row-major blocks ]
so every DMA is one fully-contiguous run per partition, and per chunk:
  - the j-major strip is reduced on the otherwise-idle TensorE as 8
    PSUM-accumulated [K=128]x[1,384] matmuls against v_col  (~3.5 us),
  - 2 row-major blocks go to DVE custom TENSOR_TENSOR_REDUCE (~1.1 us each),
  - 3 row-major blocks go to DVE fp16 tensor_mul (2x mode, ~0.6 us) +
    ScalarE accumulate-Copy (~1.4 us each),
  - the strip PSUM drain (+bias) runs on DVE (tensor_scalar_add, ~0.5 us).
Each engine needs ~3.5-4.3 us against ~4.9 us of chunk DMA, leaving margin
for the chip's intermittent ~30% DVFS throttling of engine clocks.

Output DMAs are issued from the (otherwise idle) GpSimd queue: issuing them
on the Sync queue head-of-line-blocks the enc stream behind compute.

Stage 1 (TensorE, fp16): v = W @ h via 16 PSUM-accumulated matmuls behind
chunked wt DMAs; v is PE-transposed 128 at a time into column form (v_col,
stationary operand of the strip matmuls) and partition-broadcast on the PE
(v_rep, for DVE/ScalarE).

Host-side prep is layout/dtype only (transpose/cast/reshape); all arithmetic
runs on device. The host re-assembles the three output tensors (strip rows,
TTR columns, mul+accum columns) into the full [B, N, I] output.
"""

import numpy as np

B, N, I, H = 8, 64, 128, 1024
P = 128
NI = N * I  # 8192 rows per core
KB = H // P  # 8 k blocks
N_CORES = 8
NCH = 8  # stage-2 chunks per core (2 MiB fp16 each)
SW = 384  # strip width: 3 row blocks per chunk on the PE
RM = 5  # row-major blocks per chunk on DVE/ScalarE
PATTERN = ("A", "B", "A", "B", "B")  # rm slot -> engine path
NA = sum(1 for p in PATTERN if p == "A")  # TTR cols per chunk
NB = RM - NA  # mul+accum cols per chunk

_NC_CACHE = {}
LAST_RESULTS = None


def _build(ebufs=6):
    import concourse.bacc as bacc
    import concourse.mybir as mybir
    import concourse.tile as tile
    from concourse import dve_ops

    f32 = mybir.dt.float32
    f16 = mybir.dt.float16
    Copy = mybir.ActivationFunctionType.Copy

    nc = bacc.Bacc(
        "TRN2",
        target_bir_lowering=False,
        debug=False,
        num_devices=N_CORES,
    )
    enc = nc.declare_dram_parameter("enc", [P, NCH * 8192], f16, isOutput=False)
    hh = nc.declare_dram_parameter("h", [P, B], f16, isOutput=False)
    wt = nc.declare_dram_parameter("wt", [P, H], f16, isOutput=False)
    bb = nc.declare_dram_parameter("bias", [1, 1], f32, isOutput=False)
    out_rows = nc.declare_dram_parameter("out_rows", [1, NCH * SW], f32, isOutput=True)
    out_a = nc.declare_dram_parameter("out_a", [P, NA * NCH], f32, isOutput=True)
    out_b = nc.declare_dram_parameter("out_b", [P, NB * NCH], f32, isOutput=True)
    pv_dram = nc.dram_tensor("pv_dram", [B, H], f32)
    v_dram = nc.dram_tensor("v_dram", [1, H], f32)

    with tile.TileContext(nc) as tc:
        with (
            tc.tile_pool(name="const", bufs=1) as const,
            tc.tile_pool(name="tpool", bufs=ebufs) as tpool,
            tc.tile_pool(name="rpool", bufs=ebufs) as rpool,
            tc.tile_pool(name="ppool", bufs=3) as ppool,
            tc.tile_pool(name="vpsum", bufs=1, space="PSUM") as vpsum,
            tc.tile_pool(name="spsum", bufs=3, space="PSUM") as spsum,
        ):
            # ---- stage 1: v = W @ h, W k-sharded across cores ----
            # Core c holds W.T rows [128c, 128c+128) and the matching slice
            # of every batch's hidden; it computes the partial v for ALL
            # batches over its own k-slice, then a [8,1024]-f32
            # ReduceScatter(add) hands each core its own batch's full v.
            h_col = const.tile([P, B], f16)
            nc.sync.dma_start(out=h_col[:], in_=hh[:, :])
            bias_col = const.tile([P, 1], f32)
            nc.sync.dma_start(out=bias_col[:], in_=bb[:, :].to_broadcast((P, 1)))
            bias_one = const.tile([1, 1], f32)
            nc.sync.dma_start(out=bias_one[:], in_=bb[:, :])
            wt_sb = const.tile([P, H], f16)
            nc.sync.dma_start(out=wt_sb[:], in_=wt[:, :])
            ones = const.tile([1, P], f16)
            nc.vector.memset(ones[:], 1.0)
            id1 = const.tile([1, 1], f16)
            nc.vector.memset(id1[:], 1.0)

            pv_sb = const.tile([B, H], f32)
            vps = [
                vpsum.tile([B, 512], f32, name=f"vp{jc}", tag=f"vp{jc}")
                for jc in range(H // 512)
            ]
            for jc in range(H // 512):
                nc.tensor.matmul(
                    vps[jc][:],
                    h_col[:],
                    wt_sb[:, jc * 512 : (jc + 1) * 512],
                    start=True,
                    stop=True,
                )
                nc.scalar.activation(
                    pv_sb[:, jc * 512 : (jc + 1) * 512], vps[jc][:], Copy
                )
            nc.sync.dma_start(out=pv_dram[:, :], in_=pv_sb[:])
            nc.gpsimd.collective_compute(
                "ReduceScatter",
                mybir.AluOpType.add,
                replica_groups=[list(range(N_CORES))],
                ins=[pv_dram[:, :]],
                outs=[v_dram[:, :]],
            )
            v32 = const.tile([1, H], f32)
            nc.sync.dma_start(out=v32[:], in_=v_dram[:, :])
            v_flat = const.tile([1, H], f16)
            nc.scalar.activation(v_flat[:], v32[:], Copy)
            # column form of v for the strip matmuls: v_col[p, jb] = v[jb*128+p]
            # (first: the PE strip path is the tightest engine)
            v_col = const.tile([P, KB], f16)
            for jb in range(KB):
                pt = vpsum.tile([P, 1], f16, name=f"pt{jb}", tag="pt")
                nc.tensor.transpose(
                    pt[:], v_flat[:, jb * P : (jb + 1) * P], id1[:]
                )
                nc.scalar.activation(v_col[:, jb : jb + 1], pt[:], Copy)
            # partition-broadcast v on the PE: ones[1,P].T @ v[1,512] -> [P,512]
            v_rep = const.tile([P, H], f16)
            for jc in range(H // 512):
                bc = vpsum.tile([P, 512], f32, name=f"bc{jc}", tag=f"bc{jc}")
                nc.tensor.matmul(
                    bc[:],
                    ones[:],
                    v_flat[:, jc * 512 : (jc + 1) * 512],
                    start=True,
                    stop=True,
                )
                nc.scalar.activation(
                    v_rep[:, jc * 512 : (jc + 1) * 512], bc[:], Copy
                )

            # ---- stage 2 ----
            acc_a = const.tile([P, NA * NCH], f32)
            acc_b = const.tile([P, NB * NCH], f32)
            dummy_a = const.tile([P, 1], f16)

            def rm_block(e_sl, path, col):
                if path == "A":
                    nc.vector._custom_dve(
                        dve_ops.TENSOR_TENSOR_REDUCE,
                        out=dummy_a[:].broadcast_to((P, H)),
                        in0=e_sl,
                        in1=v_rep[:],
                        s0=0.0,
                        s1=1.0,
                        accum_out=acc_a[:, col : col + 1],
                    )
                else:
                    prod = ppool.tile([P, H], f16)
                    nc.vector.tensor_mul(prod[:], e_sl, v_rep[:])
                    nc.scalar.activation(
                        prod[:], prod[:], Copy, accum_out=acc_b[:, col : col + 1]
                    )

            for ci in range(NCH):
                base = ci * 8192
                last = ci == NCH - 1
                ps = spsum.tile([1, SW], f32, name=f"ps{ci}", tag="ps")
                strip = const.tile([1, SW], f32, name=f"st{ci}", tag=f"st{ci}")
                t = tpool.tile([P, KB * SW], f16, name=f"t{ci}", tag="t")
                nc.sync.dma_start(out=t[:], in_=enc[:, base : base + KB * SW])
                if not last:
                    r = rpool.tile([P, RM * H], f16, name=f"r{ci}", tag="r")
                    nc.sync.dma_start(
                        out=r[:], in_=enc[:, base + KB * SW : base + 8192]
                    )
                    rms = [r[:, s * H : (s + 1) * H] for s in range(RM)]
                else:
                    # tapered final chunk: rm half split into two DMAs
                    ra = rpool.tile([P, 2 * H], f16, name="r7a", tag="r")
                    nc.sync.dma_start(
                        out=ra[:],
                        in_=enc[:, base + KB * SW : base + KB * SW + 2 * H],
                    )
                    rb = rpool.tile([P, 3 * H], f16, name="r7b", tag="r")
                    nc.sync.dma_start(
                        out=rb[:], in_=enc[:, base + KB * SW + 2 * H : base + 8192]
                    )
                    rms = [
                        ra[:, 0:H],
                        ra[:, H : 2 * H],
                        rb[:, 0:H],
                        rb[:, H : 2 * H],
                        rb[:, 2 * H : 3 * H],
                    ]
                for jb in range(KB):
                    nc.tensor.matmul(
                        ps[:],
                        v_col[:, jb : jb + 1],
                        t[:, jb * SW : (jb + 1) * SW],
                        start=(jb == 0),
                        stop=(jb == KB - 1),
                    )
                na = nb = 0
                for s, path in enumerate(PATTERN):
                    if path == "A":
                        rm_block(rms[s], "A", NA * ci + na)
                        na += 1
                    else:
                        rm_block(rms[s], "B", NB * ci + nb)
                        nb += 1
                # strip PSUM -> SBUF with bias, on DVE; out DMA on GpSimd
                nc.vector.tensor_scalar_add(strip[:], ps[:], bias_one[:])
                nc.gpsimd.dma_start(
                    out=out_rows[:, ci * SW : (ci + 1) * SW], in_=strip[:]
                )

            # bias + writeback of the block-accumulated columns
            head_a, head_b = NA * (NCH - 1), NB * (NCH - 1)
            nc.vector.tensor_scalar_add(
                acc_a[:, :head_a], acc_a[:, :head_a], bias_col[:]
            )
            nc.gpsimd.dma_start(out=out_a[:, :head_a], in_=acc_a[:, :head_a])
            nc.vector.tensor_scalar_add(
                acc_b[:, :head_b], acc_b[:, :head_b], bias_col[:]
            )
            nc.gpsimd.dma_start(out=out_b[:, :head_b], in_=acc_b[:, :head_b])
            nc.vector.tensor_scalar_add(
                acc_a[:, head_a:], acc_a[:, head_a:], bias_col[:]
            )
            nc.gpsimd.dma_start(out=out_a[:, head_a:], in_=acc_a[:, head_a:])
            nc.vector.tensor_scalar_add(
                acc_b[:, head_b:], acc_b[:, head_b:], bias_col[:]
            )
            nc.gpsimd.dma_start(out=out_b[:, head_b:], in_=acc_b[:, head_b:])
    nc.compile()
    return nc


def _get_nc():
    if "nc" not in _NC_CACHE:
        _NC_CACHE["nc"] = _build()
    return _NC_CACHE["nc"]


def _pack_enc(enc16_core):
    """[N*I, H] fp16 -> [P, NCH*8192]: per chunk ci, the j-major strip of
    blocks 8ci+0..2 first, then row-major blocks 8ci+3..8ci+7."""
    E = enc16_core.reshape(NCH, 8, P, H)  # [ci, slot, i, j], blk = 8ci+slot
    # strip: value(p, ci, jb*SW+r) = E[ci, r//128, r%128, jb*128+p], r in [0,SW)
    tr = E[:, 0:3].reshape(NCH, SW, KB, P).transpose(3, 0, 2, 1)  # [p, ci, jb, r]
    tr = np.ascontiguousarray(tr).reshape(P, NCH, KB * SW)
    rm = E[:, 3:8].transpose(2, 0, 1, 3).reshape(P, NCH, RM * H)  # [i, ci, slot*H+j]
    comb = np.concatenate([tr, rm], axis=2)  # [P, NCH, 8192]
    return np.ascontiguousarray(comb.reshape(P, NCH * 8192))


def kernel(hidden=None, encoder_hiddens=None, input_lengths=None, W=None, b=None):
    global LAST_RESULTS
    from concourse.bass_utils import run_bass_kernel_spmd

    hidden = np.asarray(hidden, dtype=np.float32)
    enc = np.asarray(encoder_hiddens, dtype=np.float32)
    W_ = np.asarray(W, dtype=np.float32)
    b_ = np.asarray(b, dtype=np.float32).reshape(1, 1)
    # W k-sharded: core c gets W.T rows [128c, 128c+128) and the matching
    # k-slice of every batch's hidden
    wt16 = W_.T.astype(np.float16).reshape(KB, P, H)
    h16 = hidden.astype(np.float16)  # [B, H]
    enc16 = enc.astype(np.float16)  # [B, N, I, H]

    nc = _get_nc()
    in_maps = []
    for core in range(N_CORES):
        in_maps.append(
            {
                "enc": _pack_enc(enc16[core].reshape(NI, H)),
                "h": np.ascontiguousarray(h16[:, core * P : (core + 1) * P].T),
                "wt": np.ascontiguousarray(wt16[core]),
                "bias": b_,
            }
        )
    res = run_bass_kernel_spmd(nc, in_maps, core_ids=list(range(N_CORES)))
    LAST_RESULTS = res

    out = np.empty((N_CORES, N, P), dtype=np.float32)
    for c in range(N_CORES):
        r = res.results[c]
        strips = np.asarray(r["out_rows"], dtype=np.float32).reshape(NCH, 3, P)
        a_cols = np.asarray(r["out_a"], dtype=np.float32).T.reshape(NCH, NA, P)
        b_cols = np.asarray(r["out_b"], dtype=np.float32).T.reshape(NCH, NB, P)
        O = out[c].reshape(NCH, 8, P)
        O[:, 0:3] = strips
        # rm slots 3..7 follow PATTERN = A,B,A,B,B
        O[:, 3] = a_cols[:, 0]
        O[:, 5] = a_cols[:, 1]
        O[:, 4] = b_cols[:, 0]
        O[:, 6] = b_cols[:, 1]
        O[:, 7] = b_cols[:, 2]
    return np.ascontiguousarray(out)


# revision 22
# speedup vs baseline: 1.6195x; 1.6195x over previous
"""Bass/Trainium2 kernel for nn_Bilinear (out[b,n,i] = enc[b,n,i,:] @ W @ hidden[b,:] + bias).

Sharding: data-parallel over B. 8 cores, one batch element each.

The kernel is HBM-traffic-bound (enc is 32 MiB/core in fp32), so all streamed
operands are cast to fp16 on the host (harness gate is rel_err < 2e-2; fp16
lands ~4e-4): enc 16 MiB + W 2 MiB per core, streamed at the observed
~420 GB/s per-core DMA rate.

With fp16 the stream outpaces what DVE+ScalarE alone can compute, so stage 2
is split across THREE compute engines. The host lays out each 2 MiB chunk of
enc (8 row blocks of 128) as:
  [ j-major (transposed) strip of 3 blocks | 5 row-major blocks ]
so every DMA is one fully-contiguous run per partition, and per chunk:
  - the j-major strip is reduced on the otherwise-idle TensorE as 8
    PSUM-accumulated [K=128]x[1,384] matmuls against v_col  (~3.5 us),
  - 2 row-major blocks go to DVE custom TENSOR_TENSOR_REDUCE (~1.1 us each),
  - 3 row-major blocks go to DVE fp16 tensor_mul (2x mode, ~0.6 us) +
    ScalarE accumulate-Copy (~1.4 us each),
  - the strip PSUM drain (+bias) runs on DVE (tensor_scalar_add, ~0.5 us).
Each engine needs ~3.5-4.3 us against ~4.9 us of chunk DMA, leaving margin
for the chip's intermittent ~30% DVFS throttling of engine clocks.

Output DMAs are issued from the (otherwise idle) GpSimd queue: issuing them
on the Sync queue head-of-line-blocks the enc stream behind compute.

Stage 1 (TensorE, fp16): v = W @ h via 16 PSUM-accumulated matmuls behind
chunked wt DMAs; v is PE-transposed 128 at a time into column form (v_col,
stationary operand of the strip matmuls) and partition-broadcast on the PE
(v_rep, for DVE/ScalarE).

Host-side prep is layout/dtype only (transpose/cast/reshape); all arithmetic
runs on device. The host re-assembles the three output tensors (strip rows,
TTR columns, mul+accum columns) into the full [B, N, I] output.
"""

import numpy as np

B, N, I, H = 8, 64, 128, 1024
P = 128
NI = N * I  # 8192 rows per core
KB = H // P  # 8 k blocks
N_CORES = 8
NCH = 8  # stage-2 chunks per core (2 MiB fp16 each)
SW = 384  # strip width: 3 row blocks per chunk on the PE
RM = 5  # row-major blocks per chunk on DVE/ScalarE
PATTERN = ("A", "B", "A", "B", "B")  # rm slot -> engine path
NA = sum(1 for p in PATTERN if p == "A")  # TTR cols per chunk
NB = RM - NA  # mul+accum cols per chunk

_NC_CACHE = {}
LAST_RESULTS = None


def _build(ebufs=6):
    import concourse.bacc as bacc
    import concourse.mybir as mybir
    import concourse.tile as tile
    from concourse import dve_ops

    f32 = mybir.dt.float32
    f16 = mybir.dt.float16
    Copy = mybir.ActivationFunctionType.Copy

    nc = bacc.Bacc(
        "TRN2",
        target_bir_lowering=False,
        debug=False,
        num_devices=N_CORES,
    )
    enc = nc.declare_dram_parameter("enc", [P, NCH * 8192], f16, isOutput=False)
    hh = nc.declare_dram_parameter("h", [P, KB], f16, isOutput=False)
    wt = nc.declare_dram_parameter("wt", [P, KB * H], f16, isOutput=False)
    bb = nc.declare_dram_parameter("bias", [1, 1], f32, isOutput=False)
    out_rows = nc.declare_dram_parameter("out_rows", [1, NCH * SW], f32, isOutput=True)
    out_a = nc.declare_dram_parameter("out_a", [P, NA * NCH], f32, isOutput=True)
    out_b = nc.declare_dram_parameter("out_b", [P, NB * NCH], f32, isOutput=True)

    with tile.TileContext(nc) as tc:
        with (
            tc.tile_pool(name="const", bufs=1) as const,
            tc.tile_pool(name="tpool", bufs=ebufs) as tpool,
            tc.tile_pool(name="rpool", bufs=ebufs) as rpool,
            tc.tile_pool(name="ppool", bufs=3) as ppool,
            tc.tile_pool(name="vpsum", bufs=1, space="PSUM") as vpsum,
            tc.tile_pool(name="spsum", bufs=3, space="PSUM") as spsum,
        ):
            # ---- stage 1: v[j] = sum_k wt[k,j] h[k] ----
            h_col = const.tile([P, KB], f16)
            nc.sync.dma_start(out=h_col[:], in_=hh[:, :])
            bias_col = const.tile([P, 1], f32)
            nc.sync.dma_start(out=bias_col[:], in_=bb[:, :].to_broadcast((P, 1)))
            bias_one = const.tile([1, 1], f32)
            nc.sync.dma_start(out=bias_one[:], in_=bb[:, :])
            # wt host-packed as [p, kb*H + j] = W.T[kb*128+p, j], DMA'd in
            # chunks so the stage-1 matmuls pipeline behind the stream
            wt_sb = const.tile([P, KB * H], f16)
            for kb in range(KB):
                nc.sync.dma_start(
                    out=wt_sb[:, kb * H : (kb + 1) * H],
                    in_=wt[:, kb * H : (kb + 1) * H],
                )
            ones = const.tile([1, P], f16)
            nc.vector.memset(ones[:], 1.0)
            id1 = const.tile([1, 1], f16)
            nc.vector.memset(id1[:], 1.0)

            v_flat = const.tile([1, H], f16)
            vps = [
                vpsum.tile([1, 512], f32, name=f"vp{jc}", tag=f"vp{jc}")
                for jc in range(H // 512)
            ]
            for kb in range(KB):
                for jc in range(H // 512):
                    nc.tensor.matmul(
                        vps[jc][:],
                        h_col[:, kb : kb + 1],
                        wt_sb[:, kb * H + jc * 512 : kb * H + (jc + 1) * 512],
                        start=(kb == 0),
                        stop=(kb == KB - 1),
                    )
            for jc in range(H // 512):
                nc.scalar.activation(
                    v_flat[:, jc * 512 : (jc + 1) * 512], vps[jc][:], Copy
                )
            # column form of v for the strip matmuls: v_col[p, jb] = v[jb*128+p]
            # (first: the PE strip path is the tightest engine)
            v_col = const.tile([P, KB], f16)
            for jb in range(KB):
                pt = vpsum.tile([P, 1], f16, name=f"pt{jb}", tag="pt")
                nc.tensor.transpose(
                    pt[:], v_flat[:, jb * P : (jb + 1) * P], id1[:]
                )
                nc.scalar.activation(v_col[:, jb : jb + 1], pt[:], Copy)
            # partition-broadcast v on the PE: ones[1,P].T @ v[1,512] -> [P,512]
            v_rep = const.tile([P, H], f16)
            for jc in range(H // 512):
                bc = vpsum.tile([P, 512], f32, name=f"bc{jc}", tag=f"bc{jc}")
                nc.tensor.matmul(
                    bc[:],
                    ones[:],
                    v_flat[:, jc * 512 : (jc + 1) * 512],
                    start=True,
                    stop=True,
                )
                nc.scalar.activation(
                    v_rep[:, jc * 512 : (jc + 1) * 512], bc[:], Copy
                )

            # ---- stage 2 ----
            acc_a = const.tile([P, NA * NCH], f32)
            acc_b = const.tile([P, NB * NCH], f32)
            dummy_a = const.tile([P, 1], f16)

            def rm_block(e_sl, path, col):
                if path == "A":
                    nc.vector._custom_dve(
                        dve_ops.TENSOR_TENSOR_REDUCE,
                        out=dummy_a[:].broadcast_to((P, H)),
                        in0=e_sl,
                        in1=v_rep[:],
                        s0=0.0,
                        s1=1.0,
                        accum_out=acc_a[:, col : col + 1],
                    )
                else:
                    prod = ppool.tile([P, H], f16)
                    nc.vector.tensor_mul(prod[:], e_sl, v_rep[:])
                    nc.scalar.activation(
                        prod[:], prod[:], Copy, accum_out=acc_b[:, col : col + 1]
                    )

            for ci in range(NCH):
                base = ci * 8192
                last = ci == NCH - 1
                ps = spsum.tile([1, SW], f32, name=f"ps{ci}", tag="ps")
                strip = const.tile([1, SW], f32, name=f"st{ci}", tag=f"st{ci}")
                t = tpool.tile([P, KB * SW], f16, name=f"t{ci}", tag="t")
                nc.sync.dma_start(out=t[:], in_=enc[:, base : base + KB * SW])
                if not last:
                    r = rpool.tile([P, RM * H], f16, name=f"r{ci}", tag="r")
                    nc.sync.dma_start(
                        out=r[:], in_=enc[:, base + KB * SW : base + 8192]
                    )
                    rms = [r[:, s * H : (s + 1) * H] for s in range(RM)]
                else:
                    # tapered final chunk: rm half split into two DMAs
                    ra = rpool.tile([P, 2 * H], f16, name="r7a", tag="r")
                    nc.sync.dma_start(
                        out=ra[:],
                        in_=enc[:, base + KB * SW : base + KB * SW + 2 * H],
                    )
                    rb = rpool.tile([P, 3 * H], f16, name="r7b", tag="r")
                    nc.sync.dma_start(
                        out=rb[:], in_=enc[:, base + KB * SW + 2 * H : base + 8192]
                    )
                    rms = [
                        ra[:, 0:H],
                        ra[:, H : 2 * H],
                        rb[:, 0:H],
                        rb[:, H : 2 * H],
                        rb[:, 2 * H : 3 * H],
                    ]
                for jb in range(KB):
                    nc.tensor.matmul(
                        ps[:],
                        v_col[:, jb : jb + 1],
                        t[:, jb * SW : (jb + 1) * SW],
                        start=(jb == 0),
                        stop=(jb == KB - 1),
                    )
                na = nb = 0
                for s, path in enumerate(PATTERN):
                    if path == "A":
                        rm_block(rms[s], "A", NA * ci + na)
                        na += 1
                    else:
                        rm_block(rms[s], "B", NB * ci + nb)
                        nb += 1
                # strip PSUM -> SBUF with bias, on DVE; out DMA on GpSimd
                nc.vector.tensor_scalar_add(strip[:], ps[:], bias_one[:])
                nc.gpsimd.dma_start(
                    out=out_rows[:, ci * SW : (ci + 1) * SW], in_=strip[:]
                )

            # bias + writeback of the block-accumulated columns
            head_a, head_b = NA * (NCH - 1), NB * (NCH - 1)
            nc.vector.tensor_scalar_add(
                acc_a[:, :head_a], acc_a[:, :head_a], bias_col[:]
            )
            nc.gpsimd.dma_start(out=out_a[:, :head_a], in_=acc_a[:, :head_a])
            nc.vector.tensor_scalar_add(
                acc_b[:, :head_b], acc_b[:, :head_b], bias_col[:]
            )
            nc.gpsimd.dma_start(out=out_b[:, :head_b], in_=acc_b[:, :head_b])
            nc.vector.tensor_scalar_add(
                acc_a[:, head_a:], acc_a[:, head_a:], bias_col[:]
            )
            nc.gpsimd.dma_start(out=out_a[:, head_a:], in_=acc_a[:, head_a:])
            nc.vector.tensor_scalar_add(
                acc_b[:, head_b:], acc_b[:, head_b:], bias_col[:]
            )
            nc.gpsimd.dma_start(out=out_b[:, head_b:], in_=acc_b[:, head_b:])
    nc.compile()
    return nc


def _get_nc():
    if "nc" not in _NC_CACHE:
        _NC_CACHE["nc"] = _build()
    return _NC_CACHE["nc"]


def _pack_enc(enc16_core):
    """[N*I, H] fp16 -> [P, NCH*8192]: per chunk ci, the j-major strip of
    blocks 8ci+0..2 first, then row-major blocks 8ci+3..8ci+7."""
    E = enc16_core.reshape(NCH, 8, P, H)  # [ci, slot, i, j], blk = 8ci+slot
    # strip: value(p, ci, jb*SW+r) = E[ci, r//128, r%128, jb*128+p], r in [0,SW)
    tr = E[:, 0:3].reshape(NCH, SW, KB, P).transpose(3, 0, 2, 1)  # [p, ci, jb, r]
    tr = np.ascontiguousarray(tr).reshape(P, NCH, KB * SW)
    rm = E[:, 3:8].transpose(2, 0, 1, 3).reshape(P, NCH, RM * H)  # [i, ci, slot*H+j]
    comb = np.concatenate([tr, rm], axis=2)  # [P, NCH, 8192]
    return np.ascontiguousarray(comb.reshape(P, NCH * 8192))


def kernel(hidden=None, encoder_hiddens=None, input_lengths=None, W=None, b=None):
    global LAST_RESULTS
    from concourse.bass_utils import run_bass_kernel_spmd

    hidden = np.asarray(hidden, dtype=np.float32)
    enc = np.asarray(encoder_hiddens, dtype=np.float32)
    W_ = np.asarray(W, dtype=np.float32)
    b_ = np.asarray(b, dtype=np.float32).reshape(1, 1)
    # wt packed [p, kb*H + j] = W.T[kb*128+p, j]: contiguous-run DMAs
    wt16 = np.ascontiguousarray(
        W_.T.astype(np.float16).reshape(KB, P, H).transpose(1, 0, 2).reshape(P, KB * H)
    )
    enc16 = enc.astype(np.float16)  # [B, N, I, H]

    nc = _get_nc()
    in_maps = []
    for core in range(N_CORES):
        in_maps.append(
            {
                "enc": _pack_enc(enc16[core].reshape(NI, H)),
                "h": np.ascontiguousarray(
                    hidden[core].reshape(KB, P).T.astype(np.float16)
                ),
                "wt": wt16,
                "bias": b_,
            }
        )
    res = run_bass_kernel_spmd(nc, in_maps, core_ids=list(range(N_CORES)))
    LAST_RESULTS = res

    out = np.empty((N_CORES, N, P), dtype=np.float32)
    for c in range(N_CORES):
        r = res.results[c]
        strips = np.asarray(r["out_rows"], dtype=np.float32).reshape(NCH, 3, P)
        a_cols = np.asarray(r["out_a"], dtype=np.float32).T.reshape(NCH, NA, P)
        b_cols = np.asarray(r["out_b"], dtype=np.float32).T.reshape(NCH, NB, P)
        O = out[c].reshape(NCH, 8, P)
        O[:, 0:3] = strips
        # rm slots 3..7 follow PATTERN = A,B,A,B,B
        O[:, 3] = a_cols[:, 0]
        O[:, 5] = a_cols[:, 1]
        O[:, 4] = b_cols[:, 0]
        O[:, 6] = b_cols[:, 1]
        O[:, 7] = b_cols[:, 2]
    return np.ascontiguousarray(out)


# revision 24
# speedup vs baseline: 1.7757x; 1.0964x over previous
"""Bass/Trainium2 kernel for nn_Bilinear (out[b,n,i] = enc[b,n,i,:] @ W @ hidden[b,:] + bias).

Sharding: data-parallel over B. 8 cores, one batch element each.

The kernel is HBM-traffic-bound (enc is 32 MiB/core in fp32), so all streamed
operands are cast to fp16 on the host (harness gate is rel_err < 2e-2; fp16
lands ~4e-4): enc 16 MiB + W 2 MiB per core, streamed at the observed
~420 GB/s per-core DMA rate.

With fp16 the stream outpaces what DVE+ScalarE alone can compute, so stage 2
is split across THREE compute engines. The host lays out each 2 MiB chunk of
enc (8 row blocks of 128) as:
  [ j-major (transposed) strip of 3 blocks | 5 row-major blocks ]
so every DMA is one fully-contiguous run per partition, and per chunk:
  - the j-major strip is reduced on the otherwise-idle TensorE as 8
    PSUM-accumulated [K=128]x[1,384] matmuls against v_col  (~3.5 us),
  - 2 row-major blocks go to DVE custom TENSOR_TENSOR_REDUCE (~1.1 us each),
  - 3 row-major blocks go to DVE fp16 tensor_mul (2x mode, ~0.6 us) +
    ScalarE accumulate-Copy (~1.4 us each),
  - the strip PSUM drain (+bias) runs on DVE (tensor_scalar_add, ~0.5 us).
Each engine needs ~3.5-4.3 us against ~4.9 us of chunk DMA, leaving margin
for the chip's intermittent ~30% DVFS throttling of engine clocks.

Output DMAs are issued from the (otherwise idle) GpSimd queue: issuing them
on the Sync queue head-of-line-blocks the enc stream behind compute.

Stage 1 (TensorE, fp16): v = W @ h via 16 PSUM-accumulated matmuls behind
chunked wt DMAs; v is PE-transposed 128 at a time into column form (v_col,
stationary operand of the strip matmuls) and partition-broadcast on the PE
(v_rep, for DVE/ScalarE).

Host-side prep is layout/dtype only (transpose/cast/reshape); all arithmetic
runs on device. The host re-assembles the three output tensors (strip rows,
TTR columns, mul+accum columns) into the full [B, N, I] output.
"""

import numpy as np

B, N, I, H = 8, 64, 128, 1024
P = 128
NI = N * I  # 8192 rows per core
KB = H // P  # 8 k blocks
N_CORES = 8
NCH = 8  # stage-2 chunks per core (2 MiB fp16 each)
SW = 384  # strip width: 3 row blocks per chunk on the PE
RM = 5  # row-major blocks per chunk on DVE/ScalarE
PATTERN = ("A", "B", "A", "B", "B")  # rm slot -> engine path
NA = sum(1 for p in PATTERN if p == "A")  # TTR cols per chunk
NB = RM - NA  # mul+accum cols per chunk

_NC_CACHE = {}
LAST_RESULTS = None


def _build(ebufs=7):
    import concourse.bacc as bacc
    import concourse.mybir as mybir
    import concourse.tile as tile
    from concourse import dve_ops

    f32 = mybir.dt.float32
    f16 = mybir.dt.float16
    Copy = mybir.ActivationFunctionType.Copy

    nc = bacc.Bacc(
        "TRN2",
        target_bir_lowering=False,
        debug=False,
        num_devices=N_CORES,
    )
    enc = nc.declare_dram_parameter("enc", [P, NCH * 8192], f16, isOutput=False)
    hh = nc.declare_dram_parameter("h", [P, KB + 1], f16, isOutput=False)
    wt = nc.declare_dram_parameter("wt", [P, KB * H], f16, isOutput=False)
    bb = nc.declare_dram_parameter("bias", [1, 1], f32, isOutput=False)
    out_rows = nc.declare_dram_parameter("out_rows", [1, NCH * SW], f32, isOutput=True)
    out_a = nc.declare_dram_parameter("out_a", [P, NA * NCH], f32, isOutput=True)
    out_b = nc.declare_dram_parameter("out_b", [P, NB * NCH], f32, isOutput=True)

    with tile.TileContext(nc) as tc:
        with (
            tc.tile_pool(name="const", bufs=1) as const,
            tc.tile_pool(name="tpool", bufs=ebufs) as tpool,
            tc.tile_pool(name="rpool", bufs=ebufs) as rpool,
            tc.tile_pool(name="ppool", bufs=3) as ppool,
            tc.tile_pool(name="vpsum", bufs=1, space="PSUM") as vpsum,
            tc.tile_pool(name="spsum", bufs=3, space="PSUM") as spsum,
        ):
            # ---- stage 1: v[j] = sum_k wt[k,j] h[k] ----
            h_col = const.tile([P, KB + 1], f16)
            nc.sync.dma_start(out=h_col[:], in_=hh[:, :])
            bias_col = const.tile([P, 1], f32)
            nc.scalar.activation(bias_col[:], h_col[:, KB : KB + 1], Copy)
            bias_one = bias_col[0:1, 0:1]
            # wt host-packed as [p, kb*H + j] = W.T[kb*128+p, j], DMA'd in
            # chunks so the stage-1 matmuls pipeline behind the stream
            wt_sb = const.tile([P, KB * H], f16)
            for kb in range(KB):
                nc.sync.dma_start(
                    out=wt_sb[:, kb * H : (kb + 1) * H],
                    in_=wt[:, kb * H : (kb + 1) * H],
                )
            ones = const.tile([1, P], f16)
            nc.vector.memset(ones[:], 1.0)
            id1 = const.tile([1, 1], f16)
            nc.vector.memset(id1[:], 1.0)

            v_flat = const.tile([1, H], f16)
            vps = [
                vpsum.tile([1, 512], f32, name=f"vp{jc}", tag=f"vp{jc}")
                for jc in range(H // 512)
            ]
            for kb in range(KB):
                for jc in range(H // 512):
                    nc.tensor.matmul(
                        vps[jc][:],
                        h_col[:, kb : kb + 1],
                        wt_sb[:, kb * H + jc * 512 : kb * H + (jc + 1) * 512],
                        start=(kb == 0),
                        stop=(kb == KB - 1),
                    )
            for jc in range(H // 512):
                nc.scalar.activation(
                    v_flat[:, jc * 512 : (jc + 1) * 512], vps[jc][:], Copy
                )
            # column form of v for the strip matmuls: v_col[p, jb] = v[jb*128+p]
            # (first: the PE strip path is the tightest engine)
            v_col = const.tile([P, KB], f16)
            for jb in range(KB):
                pt = vpsum.tile([P, 1], f16, name=f"pt{jb}", tag="pt")
                nc.tensor.transpose(
                    pt[:], v_flat[:, jb * P : (jb + 1) * P], id1[:]
                )
                nc.scalar.activation(v_col[:, jb : jb + 1], pt[:], Copy)
            # partition-broadcast v on the PE: ones[1,P].T @ v[1,512] -> [P,512]
            v_rep = const.tile([P, H], f16)
            for jc in range(H // 512):
                bc = vpsum.tile([P, 512], f32, name=f"bc{jc}", tag=f"bc{jc}")
                nc.tensor.matmul(
                    bc[:],
                    ones[:],
                    v_flat[:, jc * 512 : (jc + 1) * 512],
                    start=True,
                    stop=True,
                )
                nc.scalar.activation(
                    v_rep[:, jc * 512 : (jc + 1) * 512], bc[:], Copy
                )

            # ---- stage 2 ----
            acc_a = const.tile([P, NA * NCH], f32)
            acc_b = const.tile([P, NB * NCH], f32)
            dummy_a = const.tile([P, 1], f16)

            def rm_block(e_sl, path, col):
                if path == "A":
                    nc.vector._custom_dve(
                        dve_ops.TENSOR_TENSOR_REDUCE,
                        out=dummy_a[:].broadcast_to((P, H)),
                        in0=e_sl,
                        in1=v_rep[:],
                        s0=0.0,
                        s1=1.0,
                        accum_out=acc_a[:, col : col + 1],
                    )
                else:
                    prod = ppool.tile([P, H], f16)
                    nc.vector.tensor_mul(prod[:], e_sl, v_rep[:])
                    nc.scalar.activation(
                        prod[:], prod[:], Copy, accum_out=acc_b[:, col : col + 1]
                    )

            for ci in range(NCH):
                base = ci * 8192
                last = ci == NCH - 1
                ps = spsum.tile([1, SW], f32, name=f"ps{ci}", tag="ps")
                strip = const.tile([1, SW], f32, name=f"st{ci}", tag=f"st{ci}")
                t = tpool.tile([P, KB * SW], f16, name=f"t{ci}", tag="t")
                nc.sync.dma_start(out=t[:], in_=enc[:, base : base + KB * SW])
                if not last:
                    r = rpool.tile([P, RM * H], f16, name=f"r{ci}", tag="r")
                    nc.sync.dma_start(
                        out=r[:], in_=enc[:, base + KB * SW : base + 8192]
                    )
                    rms = [r[:, s * H : (s + 1) * H] for s in range(RM)]
                else:
                    # tapered final chunk: rm half split into two DMAs
                    ra = rpool.tile([P, 2 * H], f16, name="r7a", tag="r")
                    nc.sync.dma_start(
                        out=ra[:],
                        in_=enc[:, base + KB * SW : base + KB * SW + 2 * H],
                    )
                    rb = rpool.tile([P, 3 * H], f16, name="r7b", tag="r")
                    nc.sync.dma_start(
                        out=rb[:], in_=enc[:, base + KB * SW + 2 * H : base + 8192]
                    )
                    rms = [
                        ra[:, 0:H],
                        ra[:, H : 2 * H],
                        rb[:, 0:H],
                        rb[:, H : 2 * H],
                        rb[:, 2 * H : 3 * H],
                    ]
                for jb in range(KB):
                    nc.tensor.matmul(
                        ps[:],
                        v_col[:, jb : jb + 1],
                        t[:, jb * SW : (jb + 1) * SW],
                        start=(jb == 0),
                        stop=(jb == KB - 1),
                    )
                na = nb = 0
                for s, path in enumerate(PATTERN):
                    if path == "A":
                        rm_block(rms[s], "A", NA * ci + na)
                        na += 1
                    else:
                        rm_block(rms[s], "B", NB * ci + nb)
                        nb += 1
                # strip PSUM -> SBUF with bias, on DVE; out DMA on GpSimd
                nc.vector.tensor_scalar_add(strip[:], ps[:], bias_one)
                nc.gpsimd.dma_start(
                    out=out_rows[:, ci * SW : (ci + 1) * SW], in_=strip[:]
                )

            # bias + writeback of the block-accumulated columns
            head_a, head_b = NA * (NCH - 1), NB * (NCH - 1)
            nc.vector.tensor_scalar_add(
                acc_a[:, :head_a], acc_a[:, :head_a], bias_col[:]
            )
            nc.gpsimd.dma_start(out=out_a[:, :head_a], in_=acc_a[:, :head_a])
            nc.vector.tensor_scalar_add(
                acc_b[:, :head_b], acc_b[:, :head_b], bias_col[:]
            )
            nc.gpsimd.dma_start(out=out_b[:, :head_b], in_=acc_b[:, :head_b])
            nc.vector.tensor_scalar_add(
                acc_a[:, head_a:], acc_a[:, head_a:], bias_col[:]
            )
            nc.gpsimd.dma_start(out=out_a[:, head_a:], in_=acc_a[:, head_a:])
            nc.vector.tensor_scalar_add(
                acc_b[:, head_b:], acc_b[:, head_b:], bias_col[:]
            )
            nc.gpsimd.dma_start(out=out_b[:, head_b:], in_=acc_b[:, head_b:])
    nc.compile()
    return nc


def _get_nc():
    if "nc" not in _NC_CACHE:
        _NC_CACHE["nc"] = _build()
    return _NC_CACHE["nc"]


def _pack_enc(enc16_core):
    """[N*I, H] fp16 -> [P, NCH*8192]: per chunk ci, the j-major strip of
    blocks 8ci+0..2 first, then row-major blocks 8ci+3..8ci+7."""
    E = enc16_core.reshape(NCH, 8, P, H)  # [ci, slot, i, j], blk = 8ci+slot
    # strip: value(p, ci, jb*SW+r) = E[ci, r//128, r%128, jb*128+p], r in [0,SW)
    tr = E[:, 0:3].reshape(NCH, SW, KB, P).transpose(3, 0, 2, 1)  # [p, ci, jb, r]
    tr = np.ascontiguousarray(tr).reshape(P, NCH, KB * SW)
    rm = E[:, 3:8].transpose(2, 0, 1, 3).reshape(P, NCH, RM * H)  # [i, ci, slot*H+j]
    comb = np.concatenate([tr, rm], axis=2)  # [P, NCH, 8192]
    return np.ascontiguousarray(comb.reshape(P, NCH * 8192))


def kernel(hidden=None, encoder_hiddens=None, input_lengths=None, W=None, b=None):
    global LAST_RESULTS
    from concourse.bass_utils import run_bass_kernel_spmd

    hidden = np.asarray(hidden, dtype=np.float32)
    enc = np.asarray(encoder_hiddens, dtype=np.float32)
    W_ = np.asarray(W, dtype=np.float32)
    b_ = np.asarray(b, dtype=np.float32).reshape(1, 1)
    # wt packed [p, kb*H + j] = W.T[kb*128+p, j]: contiguous-run DMAs
    wt16 = np.ascontiguousarray(
        W_.T.astype(np.float16).reshape(KB, P, H).transpose(1, 0, 2).reshape(P, KB * H)
    )
    enc16 = enc.astype(np.float16)  # [B, N, I, H]

    nc = _get_nc()
    in_maps = []
    for core in range(N_CORES):
        in_maps.append(
            {
                "enc": _pack_enc(enc16[core].reshape(NI, H)),
                "h": np.ascontiguousarray(
                    np.concatenate(
                        [
                            hidden[core].reshape(KB, P).T,
                            np.full((P, 1), b_[0, 0], dtype=np.float32),
                        ],
                        axis=1,
                    ).astype(np.float16)
                ),
                "wt": wt16,
                "bias": b_,
            }
        )
    res = run_bass_kernel_spmd(nc, in_maps, core_ids=list(range(N_CORES)))
    LAST_RESULTS = res

    out = np.empty((N_CORES, N, P), dtype=np.float32)
    for c in range(N_CORES):
        r = res.results[c]
        strips = np.asarray(r["out_rows"], dtype=np.float32).reshape(NCH, 3, P)
        a_cols = np.asarray(r["out_a"], dtype=np.float32).T.reshape(NCH, NA, P)
        b_cols = np.asarray(r["out_b"], dtype=np.float32).T.reshape(NCH, NB, P)
        O = out[c].reshape(NCH, 8, P)
        O[:, 0:3] = strips
        # rm slots 3..7 follow PATTERN = A,B,A,B,B
        O[:, 3] = a_cols[:, 0]
        O[:, 5] = a_cols[:, 1]
        O[:, 4] = b_cols[:, 0]
        O[:, 6] = b_cols[:, 1]
        O[:, 7] = b_cols[:, 2]
    return np.ascontiguousarray(out)


# revision 25
# speedup vs baseline: 1.8982x; 1.0690x over previous
"""Bass/Trainium2 kernel for nn_Bilinear (out[b,n,i] = enc[b,n,i,:] @ W @ hidden[b,:] + bias).

Sharding: data-parallel over B. 8 cores, one batch element each.

The kernel is HBM-traffic-bound (enc is 32 MiB/core in fp32), so all streamed
operands are cast to fp16 on the host (harness gate is rel_err < 2e-2; fp16
lands ~4e-4): enc 16 MiB + W 2 MiB per core, streamed at the observed
~420 GB/s per-core DMA rate.

With fp16 the stream outpaces what DVE+ScalarE alone can compute, so stage 2
is split across THREE compute engines. The host lays out each 2 MiB chunk of
enc (8 row blocks of 128) as:
  [ j-major (transposed) strip of 3 blocks | 5 row-major blocks ]
so every DMA is one fully-contiguous run per partition, and per chunk:
  - the j-major strip is reduced on the otherwise-idle TensorE as 8
    PSUM-accumulated [K=128]x[1,384] matmuls against v_col  (~3.5 us),
  - 2 row-major blocks go to DVE custom TENSOR_TENSOR_REDUCE (~1.1 us each),
  - 3 row-major blocks go to DVE fp16 tensor_mul (2x mode, ~0.6 us) +
    ScalarE accumulate-Copy (~1.4 us each),
  - the strip PSUM drain (+bias) runs on DVE (tensor_scalar_add, ~0.5 us).
Each engine needs ~3.5-4.3 us against ~4.9 us of chunk DMA, leaving margin
for the chip's intermittent ~30% DVFS throttling of engine clocks.

Output DMAs are issued from the (otherwise idle) GpSimd queue: issuing them
on the Sync queue head-of-line-blocks the enc stream behind compute.

Stage 1 (TensorE, fp16): v = W @ h via 16 PSUM-accumulated matmuls behind
chunked wt DMAs; v is PE-transposed 128 at a time into column form (v_col,
stationary operand of the strip matmuls) and partition-broadcast on the PE
(v_rep, for DVE/ScalarE).

Host-side prep is layout/dtype only (transpose/cast/reshape); all arithmetic
runs on device. The host re-assembles the three output tensors (strip rows,
TTR columns, mul+accum columns) into the full [B, N, I] output.
"""

import numpy as np

B, N, I, H = 8, 64, 128, 1024
P = 128
NI = N * I  # 8192 rows per core
KB = H // P  # 8 k blocks
N_CORES = 8
NCH = 8  # stage-2 chunks per core (2 MiB fp16 each)
SW = 384  # strip width: 3 row blocks per chunk on the PE
RM = 5  # row-major blocks per chunk on DVE/ScalarE
PATTERN = ("A", "B", "A", "B", "B")  # rm slot -> engine path
NA = sum(1 for p in PATTERN if p == "A")  # TTR cols per chunk
NB = RM - NA  # mul+accum cols per chunk

_NC_CACHE = {}
LAST_RESULTS = None


def _build(ebufs=8):
    import concourse.bacc as bacc
    import concourse.mybir as mybir
    import concourse.tile as tile
    from concourse import dve_ops

    f32 = mybir.dt.float32
    f16 = mybir.dt.float16
    Copy = mybir.ActivationFunctionType.Copy

    nc = bacc.Bacc(
        "TRN2",
        target_bir_lowering=False,
        debug=False,
        num_devices=N_CORES,
    )
    enc = nc.declare_dram_parameter("enc", [P, NCH * 8192], f16, isOutput=False)
    hh = nc.declare_dram_parameter("h", [P, KB + 1], f16, isOutput=False)
    wt = nc.declare_dram_parameter("wt", [P, KB * H], f16, isOutput=False)
    bb = nc.declare_dram_parameter("bias", [1, 1], f32, isOutput=False)
    out_rows = nc.declare_dram_parameter("out_rows", [1, NCH * SW], f32, isOutput=True)
    out_a = nc.declare_dram_parameter("out_a", [P, NA * NCH], f32, isOutput=True)
    out_b = nc.declare_dram_parameter("out_b", [P, NB * NCH], f32, isOutput=True)

    with tile.TileContext(nc) as tc:
        with (
            tc.tile_pool(name="const", bufs=1) as const,
            tc.tile_pool(name="tpool", bufs=ebufs) as tpool,
            tc.tile_pool(name="rpool", bufs=ebufs) as rpool,
            tc.tile_pool(name="ppool", bufs=3) as ppool,
            tc.tile_pool(name="vpsum", bufs=1, space="PSUM") as vpsum,
            tc.tile_pool(name="spsum", bufs=3, space="PSUM") as spsum,
        ):
            # ---- stage 1: v[j] = sum_k wt[k,j] h[k] ----
            h_col = const.tile([P, KB + 1], f16)
            nc.sync.dma_start(out=h_col[:], in_=hh[:, :])
            bias_col = const.tile([P, 1], f32)
            nc.scalar.activation(bias_col[:], h_col[:, KB : KB + 1], Copy)
            bias_one = bias_col[0:1, 0:1]
            # wt host-packed as [p, kb*H + j] = W.T[kb*128+p, j], DMA'd in
            # chunks so the stage-1 matmuls pipeline behind the stream
            wt_sb = const.tile([P, KB * H], f16)
            for kb in range(KB):
                nc.sync.dma_start(
                    out=wt_sb[:, kb * H : (kb + 1) * H],
                    in_=wt[:, kb * H : (kb + 1) * H],
                )
            ones = const.tile([1, P], f16)
            nc.vector.memset(ones[:], 1.0)
            id1 = const.tile([1, 1], f16)
            nc.vector.memset(id1[:], 1.0)

            v_flat = const.tile([1, H], f16)
            vps = [
                vpsum.tile([1, 512], f32, name=f"vp{jc}", tag=f"vp{jc}")
                for jc in range(H // 512)
            ]
            for kb in range(KB):
                for jc in range(H // 512):
                    nc.tensor.matmul(
                        vps[jc][:],
                        h_col[:, kb : kb + 1],
                        wt_sb[:, kb * H + jc * 512 : kb * H + (jc + 1) * 512],
                        start=(kb == 0),
                        stop=(kb == KB - 1),
                    )
            for jc in range(H // 512):
                nc.scalar.activation(
                    v_flat[:, jc * 512 : (jc + 1) * 512], vps[jc][:], Copy
                )
            # column form of v for the strip matmuls: v_col[p, jb] = v[jb*128+p]
            # (first: the PE strip path is the tightest engine)
            v_col = const.tile([P, KB], f16)
            for jb in range(KB):
                pt = vpsum.tile([P, 1], f16, name=f"pt{jb}", tag="pt")
                nc.tensor.transpose(
                    pt[:], v_flat[:, jb * P : (jb + 1) * P], id1[:]
                )
                nc.scalar.activation(v_col[:, jb : jb + 1], pt[:], Copy)
            # partition-broadcast v on the PE: ones[1,P].T @ v[1,512] -> [P,512]
            v_rep = const.tile([P, H], f16)
            for jc in range(H // 512):
                bc = vpsum.tile([P, 512], f32, name=f"bc{jc}", tag=f"bc{jc}")
                nc.tensor.matmul(
                    bc[:],
                    ones[:],
                    v_flat[:, jc * 512 : (jc + 1) * 512],
                    start=True,
                    stop=True,
                )
                nc.scalar.activation(
                    v_rep[:, jc * 512 : (jc + 1) * 512], bc[:], Copy
                )

            # ---- stage 2 ----
            acc_a = const.tile([P, NA * NCH], f32)
            acc_b = const.tile([P, NB * NCH], f32)
            dummy_a = const.tile([P, 1], f16)

            def rm_block(e_sl, path, col):
                if path == "A":
                    nc.vector._custom_dve(
                        dve_ops.TENSOR_TENSOR_REDUCE,
                        out=dummy_a[:].broadcast_to((P, H)),
                        in0=e_sl,
                        in1=v_rep[:],
                        s0=0.0,
                        s1=1.0,
                        accum_out=acc_a[:, col : col + 1],
                    )
                else:
                    prod = ppool.tile([P, H], f16)
                    nc.vector.tensor_mul(prod[:], e_sl, v_rep[:])
                    nc.scalar.activation(
                        prod[:], prod[:], Copy, accum_out=acc_b[:, col : col + 1]
                    )

            head_a, head_b = NA * (NCH - 1), NB * (NCH - 1)
            for ci in range(NCH):
                base = ci * 8192
                last = ci == NCH - 1
                if last:
                    # head columns are final: bias + writeback now, so only
                    # the last chunk's few columns stay in the tail
                    nc.vector.tensor_scalar_add(
                        acc_a[:, :head_a], acc_a[:, :head_a], bias_col[:]
                    )
                    nc.gpsimd.dma_start(
                        out=out_a[:, :head_a], in_=acc_a[:, :head_a]
                    )
                    nc.vector.tensor_scalar_add(
                        acc_b[:, :head_b], acc_b[:, :head_b], bias_col[:]
                    )
                    nc.gpsimd.dma_start(
                        out=out_b[:, :head_b], in_=acc_b[:, :head_b]
                    )
                ps = spsum.tile([1, SW], f32, name=f"ps{ci}", tag="ps")
                strip = const.tile([1, SW], f32, name=f"st{ci}", tag=f"st{ci}")
                t = tpool.tile([P, KB * SW], f16, name=f"t{ci}", tag="t")
                nc.sync.dma_start(out=t[:], in_=enc[:, base : base + KB * SW])
                if not last:
                    r = rpool.tile([P, RM * H], f16, name=f"r{ci}", tag="r")
                    nc.sync.dma_start(
                        out=r[:], in_=enc[:, base + KB * SW : base + 8192]
                    )
                    rms = [r[:, s * H : (s + 1) * H] for s in range(RM)]
                else:
                    # tapered final chunk: rm half split into two DMAs
                    ra = rpool.tile([P, 2 * H], f16, name="r7a", tag="r")
                    nc.sync.dma_start(
                        out=ra[:],
                        in_=enc[:, base + KB * SW : base + KB * SW + 2 * H],
                    )
                    rb = rpool.tile([P, 3 * H], f16, name="r7b", tag="r")
                    nc.sync.dma_start(
                        out=rb[:], in_=enc[:, base + KB * SW + 2 * H : base + 8192]
                    )
                    rms = [
                        ra[:, 0:H],
                        ra[:, H : 2 * H],
                        rb[:, 0:H],
                        rb[:, H : 2 * H],
                        rb[:, 2 * H : 3 * H],
                    ]
                for jb in range(KB):
                    nc.tensor.matmul(
                        ps[:],
                        v_col[:, jb : jb + 1],
                        t[:, jb * SW : (jb + 1) * SW],
                        start=(jb == 0),
                        stop=(jb == KB - 1),
                    )
                na = nb = 0
                for s, path in enumerate(PATTERN):
                    if path == "A":
                        rm_block(rms[s], "A", NA * ci + na)
                        na += 1
                    else:
                        rm_block(rms[s], "B", NB * ci + nb)
                        nb += 1
                # strip PSUM -> SBUF with bias, on DVE; out DMA on GpSimd
                nc.vector.tensor_scalar_add(strip[:], ps[:], bias_one)
                nc.gpsimd.dma_start(
                    out=out_rows[:, ci * SW : (ci + 1) * SW], in_=strip[:]
                )

            # tail columns of the block-accumulated outputs
            nc.vector.tensor_scalar_add(
                acc_a[:, head_a:], acc_a[:, head_a:], bias_col[:]
            )
            nc.gpsimd.dma_start(out=out_a[:, head_a:], in_=acc_a[:, head_a:])
            nc.vector.tensor_scalar_add(
                acc_b[:, head_b:], acc_b[:, head_b:], bias_col[:]
            )
            nc.gpsimd.dma_start(out=out_b[:, head_b:], in_=acc_b[:, head_b:])
    nc.compile()
    return nc


def _get_nc():
    if "nc" not in _NC_CACHE:
        _NC_CACHE["nc"] = _build()
    return _NC_CACHE["nc"]


def _pack_enc(enc16_core):
    """[N*I, H] fp16 -> [P, NCH*8192]: per chunk ci, the j-major strip of
    blocks 8ci+0..2 first, then row-major blocks 8ci+3..8ci+7."""
    E = enc16_core.reshape(NCH, 8, P, H)  # [ci, slot, i, j], blk = 8ci+slot
    # strip: value(p, ci, jb*SW+r) = E[ci, r//128, r%128, jb*128+p], r in [0,SW)
    tr = E[:, 0:3].reshape(NCH, SW, KB, P).transpose(3, 0, 2, 1)  # [p, ci, jb, r]
    tr = np.ascontiguousarray(tr).reshape(P, NCH, KB * SW)
    rm = E[:, 3:8].transpose(2, 0, 1, 3).reshape(P, NCH, RM * H)  # [i, ci, slot*H+j]
    comb = np.concatenate([tr, rm], axis=2)  # [P, NCH, 8192]
    return np.ascontiguousarray(comb.reshape(P, NCH * 8192))


def kernel(hidden=None, encoder_hiddens=None, input_lengths=None, W=None, b=None):
    global LAST_RESULTS
    from concourse.bass_utils import run_bass_kernel_spmd

    hidden = np.asarray(hidden, dtype=np.float32)
    enc = np.asarray(encoder_hiddens, dtype=np.float32)
    W_ = np.asarray(W, dtype=np.float32)
    b_ = np.asarray(b, dtype=np.float32).reshape(1, 1)
    # wt packed [p, kb*H + j] = W.T[kb*128+p, j]: contiguous-run DMAs
    wt16 = np.ascontiguousarray(
        W_.T.astype(np.float16).reshape(KB, P, H).transpose(1, 0, 2).reshape(P, KB * H)
    )
    enc16 = enc.astype(np.float16)  # [B, N, I, H]

    nc = _get_nc()
    in_maps = []
    for core in range(N_CORES):
        in_maps.append(
            {
                "enc": _pack_enc(enc16[core].reshape(NI, H)),
                "h": np.ascontiguousarray(
                    np.concatenate(
                        [
                            hidden[core].reshape(KB, P).T,
                            np.full((P, 1), b_[0, 0], dtype=np.float32),
                        ],
                        axis=1,
                    ).astype(np.float16)
                ),
                "wt": wt16,
                "bias": b_,
            }
        )
    res = run_bass_kernel_spmd(nc, in_maps, core_ids=list(range(N_CORES)))
    LAST_RESULTS = res

    out = np.empty((N_CORES, N, P), dtype=np.float32)
    for c in range(N_CORES):
        r = res.results[c]
        strips = np.asarray(r["out_rows"], dtype=np.float32).reshape(NCH, 3, P)
        a_cols = np.asarray(r["out_a"], dtype=np.float32).T.reshape(NCH, NA, P)
        b_cols = np.asarray(r["out_b"], dtype=np.float32).T.reshape(NCH, NB, P)
        O = out[c].reshape(NCH, 8, P)
        O[:, 0:3] = strips
        # rm slots 3..7 follow PATTERN = A,B,A,B,B
        O[:, 3] = a_cols[:, 0]
        O[:, 5] = a_cols[:, 1]
        O[:, 4] = b_cols[:, 0]
        O[:, 6] = b_cols[:, 1]
        O[:, 7] = b_cols[:, 2]
    return np.ascontiguousarray(out)


# revision 28
# speedup vs baseline: 1.9536x; 1.0292x over previous
"""Bass/Trainium2 kernel for nn_Bilinear (out[b,n,i] = enc[b,n,i,:] @ W @ hidden[b,:] + bias).

Sharding: data-parallel over B. 8 cores, one batch element each.

The kernel is HBM-traffic-bound (enc is 32 MiB/core in fp32), so all streamed
operands are cast to fp16 on the host (harness gate is rel_err < 2e-2; fp16
lands ~4e-4): enc 16 MiB + W 2 MiB per core, streamed at the observed
~420 GB/s per-core DMA rate.

With fp16 the stream outpaces what DVE+ScalarE alone can compute, so stage 2
is split across THREE compute engines. The host lays out each 2 MiB chunk of
enc (8 row blocks of 128) as:
  [ j-major (transposed) strip of 3 blocks | 5 row-major blocks ]
so every DMA is one fully-contiguous run per partition, and per chunk:
  - the j-major strip is reduced on the otherwise-idle TensorE as 8
    PSUM-accumulated [K=128]x[1,384] matmuls against v_col  (~3.5 us),
  - 2 row-major blocks go to DVE custom TENSOR_TENSOR_REDUCE (~1.1 us each),
  - 3 row-major blocks go to DVE fp16 tensor_mul (2x mode, ~0.6 us) +
    ScalarE accumulate-Copy (~1.4 us each),
  - the strip PSUM drain (+bias) runs on DVE (tensor_scalar_add, ~0.5 us).
Each engine needs ~3.5-4.3 us against ~4.9 us of chunk DMA, leaving margin
for the chip's intermittent ~30% DVFS throttling of engine clocks.

Output DMAs are issued from the (otherwise idle) GpSimd queue: issuing them
on the Sync queue head-of-line-blocks the enc stream behind compute.

Stage 1 (TensorE, fp16): v = W @ h via 16 PSUM-accumulated matmuls behind
chunked wt DMAs; v is PE-transposed 128 at a time into column form (v_col,
stationary operand of the strip matmuls) and partition-broadcast on the PE
(v_rep, for DVE/ScalarE).

Host-side prep is layout/dtype only (transpose/cast/reshape); all arithmetic
runs on device. The host re-assembles the three output tensors (strip rows,
TTR columns, mul+accum columns) into the full [B, N, I] output.
"""

import numpy as np

B, N, I, H = 8, 64, 128, 1024
P = 128
NI = N * I  # 8192 rows per core
KB = H // P  # 8 k blocks
N_CORES = 8
NCH = 8  # stage-2 chunks per core (2 MiB fp16 each)
SW = 384  # strip width: 3 row blocks per chunk on the PE
RM = 5  # row-major blocks per chunk on DVE/ScalarE
PATTERN = ("A", "B", "A", "B", "B")  # rm slot -> engine path
NA = sum(1 for p in PATTERN if p == "A")  # TTR cols per chunk
NB = RM - NA  # mul+accum cols per chunk

_NC_CACHE = {}
LAST_RESULTS = None


def _build(ebufs=7):
    import concourse.bacc as bacc
    import concourse.mybir as mybir
    import concourse.tile as tile
    from concourse import dve_ops

    f32 = mybir.dt.float32
    f16 = mybir.dt.float16
    Copy = mybir.ActivationFunctionType.Copy

    nc = bacc.Bacc(
        "TRN2",
        target_bir_lowering=False,
        debug=False,
        num_devices=N_CORES,
    )
    enc = nc.declare_dram_parameter("enc", [P, NCH * 8192], f16, isOutput=False)
    hh = nc.declare_dram_parameter("h", [P, KB + 1], f16, isOutput=False)
    wt = nc.declare_dram_parameter("wt", [P, KB * H], f16, isOutput=False)
    bb = nc.declare_dram_parameter("bias", [1, 1], f32, isOutput=False)
    out_rows = nc.declare_dram_parameter("out_rows", [1, NCH * SW], f32, isOutput=True)
    out_a = nc.declare_dram_parameter("out_a", [P, NA * NCH], f32, isOutput=True)
    out_b = nc.declare_dram_parameter("out_b", [P, NB * NCH], f32, isOutput=True)

    with tile.TileContext(nc) as tc:
        with (
            tc.tile_pool(name="const", bufs=1) as const,
            tc.tile_pool(name="tpool", bufs=ebufs) as tpool,
            tc.tile_pool(name="rpool", bufs=ebufs) as rpool,
            tc.tile_pool(name="ppool", bufs=3) as ppool,
            tc.tile_pool(name="vpsum", bufs=1, space="PSUM") as vpsum,
            tc.tile_pool(name="spsum", bufs=3, space="PSUM") as spsum,
        ):
            # ---- stage 1: v[j] = sum_k wt[k,j] h[k] ----
            h_col = const.tile([P, KB + 1], f16)
            nc.sync.dma_start(out=h_col[:], in_=hh[:, :])
            bias_col = const.tile([P, 1], f32)
            nc.scalar.activation(bias_col[:], h_col[:, KB : KB + 1], Copy)
            bias_one = bias_col[0:1, 0:1]
            # wt host-packed as [p, kb*H + j] = W.T[kb*128+p, j], DMA'd in
            # chunks so the stage-1 matmuls pipeline behind the stream
            wt_sb = const.tile([P, KB * H], f16)
            for wc in range(4):
                nc.sync.dma_start(
                    out=wt_sb[:, wc * 2 * H : (wc + 1) * 2 * H],
                    in_=wt[:, wc * 2 * H : (wc + 1) * 2 * H],
                )
            ones = const.tile([1, P], f16)
            nc.vector.memset(ones[:], 1.0)
            id1 = const.tile([1, 1], f16)
            nc.vector.memset(id1[:], 1.0)

            v_flat = const.tile([1, H], f16)
            vps = [
                vpsum.tile([1, 512], f32, name=f"vp{jc}", tag=f"vp{jc}")
                for jc in range(H // 512)
            ]
            for kb in range(KB):
                for jc in range(H // 512):
                    nc.tensor.matmul(
                        vps[jc][:],
                        h_col[:, kb : kb + 1],
                        wt_sb[:, kb * H + jc * 512 : kb * H + (jc + 1) * 512],
                        start=(kb == 0),
                        stop=(kb == KB - 1),
                    )
            for jc in range(H // 512):
                nc.scalar.activation(
                    v_flat[:, jc * 512 : (jc + 1) * 512], vps[jc][:], Copy
                )
            # column form of v for the strip matmuls: v_col[p, jb] = v[jb*128+p]
            # (first: the PE strip path is the tightest engine)
            v_col = const.tile([P, KB], f16)
            for jb in range(KB):
                pt = vpsum.tile([P, 1], f16, name=f"pt{jb}", tag="pt")
                nc.tensor.transpose(
                    pt[:], v_flat[:, jb * P : (jb + 1) * P], id1[:]
                )
                nc.scalar.activation(v_col[:, jb : jb + 1], pt[:], Copy)
            # partition-broadcast v on the PE: ones[1,P].T @ v[1,512] -> [P,512]
            v_rep = const.tile([P, H], f16)
            for jc in range(H // 512):
                bc = vpsum.tile([P, 512], f32, name=f"bc{jc}", tag=f"bc{jc}")
                nc.tensor.matmul(
                    bc[:],
                    ones[:],
                    v_flat[:, jc * 512 : (jc + 1) * 512],
                    start=True,
                    stop=True,
                )
                nc.scalar.activation(
                    v_rep[:, jc * 512 : (jc + 1) * 512], bc[:], Copy
                )

            # ---- stage 2 ----
            acc_a = const.tile([P, NA * NCH], f32)
            acc_b = const.tile([P, NB * NCH], f32)
            dummy_a = const.tile([P, 1], f16)

            def rm_block(e_sl, path, col):
                if path == "A":
                    nc.vector._custom_dve(
                        dve_ops.TENSOR_TENSOR_REDUCE,
                        out=dummy_a[:].broadcast_to((P, H)),
                        in0=e_sl,
                        in1=v_rep[:],
                        s0=0.0,
                        s1=1.0,
                        accum_out=acc_a[:, col : col + 1],
                    )
                else:
                    prod = ppool.tile([P, H], f16)
                    nc.vector.tensor_mul(prod[:], e_sl, v_rep[:])
                    nc.scalar.activation(
                        prod[:], prod[:], Copy, accum_out=acc_b[:, col : col + 1]
                    )

            head_a, head_b = NA * (NCH - 1), NB * (NCH - 1)
            for ci in range(NCH):
                base = ci * 8192
                last = ci == NCH - 1
                if last:
                    # head columns are final: bias + writeback now, so only
                    # the last chunk's few columns stay in the tail
                    nc.vector.tensor_scalar_add(
                        acc_a[:, :head_a], acc_a[:, :head_a], bias_col[:]
                    )
                    nc.gpsimd.dma_start(
                        out=out_a[:, :head_a], in_=acc_a[:, :head_a]
                    )
                    nc.vector.tensor_scalar_add(
                        acc_b[:, :head_b], acc_b[:, :head_b], bias_col[:]
                    )
                    nc.gpsimd.dma_start(
                        out=out_b[:, :head_b], in_=acc_b[:, :head_b]
                    )
                ps = spsum.tile([1, SW], f32, name=f"ps{ci}", tag="ps")
                strip = const.tile([1, SW], f32, name=f"st{ci}", tag=f"st{ci}")
                if not last:
                    # one 16 KiB-per-partition DMA per bulk chunk: largest
                    # contiguous runs give the best per-packet DMA rate
                    e = tpool.tile([P, 8192], f16, name=f"e{ci}", tag="t")
                    nc.sync.dma_start(out=e[:], in_=enc[:, base : base + 8192])
                    tsl = [e[:, jb * SW : (jb + 1) * SW] for jb in range(KB)]
                    rms = [
                        e[:, KB * SW + s * H : KB * SW + (s + 1) * H]
                        for s in range(RM)
                    ]
                else:
                    # tapered final chunk: four half-size DMAs
                    ta = tpool.tile([P, 4 * SW], f16, name="t7a", tag="t7", bufs=2)
                    nc.sync.dma_start(out=ta[:], in_=enc[:, base : base + 4 * SW])
                    tb = tpool.tile([P, 4 * SW], f16, name="t7b", tag="t7", bufs=2)
                    nc.sync.dma_start(
                        out=tb[:], in_=enc[:, base + 4 * SW : base + KB * SW]
                    )
                    tsl = [
                        (ta if jb < 4 else tb)[:, (jb % 4) * SW : (jb % 4 + 1) * SW]
                        for jb in range(KB)
                    ]
                    ra = rpool.tile([P, 2 * H], f16, name="r7a", tag="r", bufs=2)
                    nc.sync.dma_start(
                        out=ra[:],
                        in_=enc[:, base + KB * SW : base + KB * SW + 2 * H],
                    )
                    rb = rpool.tile([P, 3 * H], f16, name="r7b", tag="r", bufs=2)
                    nc.sync.dma_start(
                        out=rb[:], in_=enc[:, base + KB * SW + 2 * H : base + 8192]
                    )
                    rms = [
                        ra[:, 0:H],
                        ra[:, H : 2 * H],
                        rb[:, 0:H],
                        rb[:, H : 2 * H],
                        rb[:, 2 * H : 3 * H],
                    ]
                for jb in range(KB):
                    nc.tensor.matmul(
                        ps[:],
                        v_col[:, jb : jb + 1],
                        tsl[jb],
                        start=(jb == 0),
                        stop=(jb == KB - 1),
                    )
                na = nb = 0
                for s, path in enumerate(PATTERN):
                    if path == "A":
                        rm_block(rms[s], "A", NA * ci + na)
                        na += 1
                    else:
                        rm_block(rms[s], "B", NB * ci + nb)
                        nb += 1
                # strip PSUM -> SBUF with bias, on DVE; out DMA on GpSimd
                nc.vector.tensor_scalar_add(strip[:], ps[:], bias_one)
                nc.gpsimd.dma_start(
                    out=out_rows[:, ci * SW : (ci + 1) * SW], in_=strip[:]
                )

            # tail columns of the block-accumulated outputs
            nc.vector.tensor_scalar_add(
                acc_a[:, head_a:], acc_a[:, head_a:], bias_col[:]
            )
            nc.gpsimd.dma_start(out=out_a[:, head_a:], in_=acc_a[:, head_a:])
            nc.vector.tensor_scalar_add(
                acc_b[:, head_b:], acc_b[:, head_b:], bias_col[:]
            )
            nc.gpsimd.dma_start(out=out_b[:, head_b:], in_=acc_b[:, head_b:])
    nc.compile()
    return nc


def _get_nc():
    if "nc" not in _NC_CACHE:
        _NC_CACHE["nc"] = _build()
    return _NC_CACHE["nc"]


def _pack_enc(enc16_core):
    """[N*I, H] fp16 -> [P, NCH*8192]: per chunk ci, the j-major strip of
    blocks 8ci+0..2 first, then row-major blocks 8ci+3..8ci+7."""
    E = enc16_core.reshape(NCH, 8, P, H)  # [ci, slot, i, j], blk = 8ci+slot
    # strip: value(p, ci, jb*SW+r) = E[ci, r//128, r%128, jb*128+p], r in [0,SW)
    tr = E[:, 0:3].reshape(NCH, SW, KB, P).transpose(3, 0, 2, 1)  # [p, ci, jb, r]
    tr = np.ascontiguousarray(tr).reshape(P, NCH, KB * SW)
    rm = E[:, 3:8].transpose(2, 0, 1, 3).reshape(P, NCH, RM * H)  # [i, ci, slot*H+j]
    comb = np.concatenate([tr, rm], axis=2)  # [P, NCH, 8192]
    return np.ascontiguousarray(comb.reshape(P, NCH * 8192))


def kernel(hidden=None, encoder_hiddens=None, input_lengths=None, W=None, b=None):
    global LAST_RESULTS
    from concourse.bass_utils import run_bass_kernel_spmd

    hidden = np.asarray(hidden, dtype=np.float32)
    enc = np.asarray(encoder_hiddens, dtype=np.float32)
    W_ = np.asarray(W, dtype=np.float32)
    b_ = np.asarray(b, dtype=np.float32).reshape(1, 1)
    # wt packed [p, kb*H + j] = W.T[kb*128+p, j]: contiguous-run DMAs
    wt16 = np.ascontiguousarray(
        W_.T.astype(np.float16).reshape(KB, P, H).transpose(1, 0, 2).reshape(P, KB * H)
    )
    enc16 = enc.astype(np.float16)  # [B, N, I, H]

    nc = _get_nc()
    in_maps = []
    for core in range(N_CORES):
        in_maps.append(
            {
                "enc": _pack_enc(enc16[core].reshape(NI, H)),
                "h": np.ascontiguousarray(
                    np.concatenate(
                        [
                            hidden[core].reshape(KB, P).T,
                            np.full((P, 1), b_[0, 0], dtype=np.float32),
                        ],
                        axis=1,
                    ).astype(np.float16)
                ),
                "wt": wt16,
                "bias": b_,
            }
        )
    res = run_bass_kernel_spmd(nc, in_maps, core_ids=list(range(N_CORES)))
    LAST_RESULTS = res

    out = np.empty((N_CORES, N, P), dtype=np.float32)
    for c in range(N_CORES):
        r = res.results[c]
        strips = np.asarray(r["out_rows"], dtype=np.float32).reshape(NCH, 3, P)
        a_cols = np.asarray(r["out_a"], dtype=np.float32).T.reshape(NCH, NA, P)
        b_cols = np.asarray(r["out_b"], dtype=np.float32).T.reshape(NCH, NB, P)
        O = out[c].reshape(NCH, 8, P)
        O[:, 0:3] = strips
        # rm slots 3..7 follow PATTERN = A,B,A,B,B
        O[:, 3] = a_cols[:, 0]
        O[:, 5] = a_cols[:, 1]
        O[:, 4] = b_cols[:, 0]
        O[:, 6] = b_cols[:, 1]
        O[:, 7] = b_cols[:, 2]
    return np.ascontiguousarray(out)
